# revision 1
# baseline (speedup 1.0000x reference)
"""Trainium2 Bass kernel for nn_CvxMPC: finite-horizon LQR gain (Riccati
recursion) + batch control computation  u0 = -obs @ K0.T.

Sharding: obs is split along batch across 8 cores (data parallel); A, B and
the entire Riccati recursion are replicated on every core (no collectives).

Device algorithm per core (all matmuls fp32r = fp32 with 11-bit mantissa
operands, fp32 PSUM accumulate; PE computes lhsT.T @ rhs):
    P = Q
    repeat 49x:
        W  = B'P            (lhsT = B)
        WT = W.T            (PE transpose)  == P B
        S  = R + WT'B
        Y  = WT'A           == B'PA
        X ~= S^-1           (Newton-Schulz, warm started across steps)
        T1 = (-X) Y
        G  = P'A == PA      (P symmetric)
        P' = Q + A'G + Y'T1 (PSUM accumulation; == Q + A'PA - Y'XY)
    K0 = X Y (+ one Newton refinement);  u0.T = -K0 @ obs.T
obs.T is built with PE transposes interleaved into the Riccati stream.
"""
import numpy as np
import concourse.bacc as bacc
import concourse.mybir as mybir
import concourse.tile as tile
from concourse import bass_utils

f32 = mybir.dt.float32
f32r = mybir.dt.float32r

N = 512          # state dim
M = 128          # control dim
STEPS = 49       # HORIZON - 1
Q_COST = 0.01
R_COST = 0.01
BATCH = 32768
NCORES = 8
SHARD = BATCH // NCORES          # 4096 rows per core
CHUNKS = SHARD // 128            # 32 [128,512] obs row-chunks per core
KT_ = N // 128                   # 4 k-tiles
OGROUPS = 8                      # obs DMA groups (4 chunks each)
OG_CH = CHUNKS // OGROUPS        # 4


def newton_iters(t):
    """Newton-Schulz iteration schedule (prototype-validated with margin)."""
    if t == 0:
        return 12
    if t < 4:
        return 4
    if t < 10:
        return 3
    if t < 20:
        return 2
    return 1


def r32r_rne(x):
    """Round fp32 -> fp32r (11-bit mantissa), round-to-nearest-even.
    Matches the DVE f32->f32r cast measured on hardware."""
    u = np.ascontiguousarray(x, np.float32).view(np.uint32).copy()
    bias = np.uint32(0x7FF) + ((u >> np.uint32(12)) & np.uint32(1))
    u = (u + bias) & np.uint32(0xFFFFF000)
    return u.view(np.float32)


# ---- constant blob layout (per-partition f32 elements) ----
OFF_A = 0                      # A  [4 x 512]  k-partition tiles, fp32r
OFF_B = OFF_A + KT_ * N        # B  [4 x 128]
OFF_P0 = OFF_B + KT_ * M       # initial P = Q  [4 x 512]
OFF_I = OFF_P0 + KT_ * N       # identity [128]
OFF_2I = OFF_I + M             # 2*I [128]
OFF_X0 = OFF_2I + M            # X0 = 25*I [128]
OFF_RD = OFF_X0 + M            # R diag = 0.01*I [128]
OFF_QR = OFF_RD + M            # Q row tiles [4 x 512] (0.01*I block at i)
CBLOB = OFF_QR + KT_ * N


def pack_k_tiles(x, width):
    """[512, width] -> [128, 4*width] with k-partition tiling."""
    return np.ascontiguousarray(
        x.reshape(KT_, 128, width).transpose(1, 0, 2).reshape(128, KT_ * width))


def build_const_blob(A, B):
    blob = np.zeros((128, CBLOB), np.float32)
    blob[:, OFF_A:OFF_A + KT_ * N] = pack_k_tiles(r32r_rne(A), N)
    blob[:, OFF_B:OFF_B + KT_ * M] = pack_k_tiles(r32r_rne(B), M)
    P0 = r32r_rne(Q_COST * np.eye(N, dtype=np.float32))
    blob[:, OFF_P0:OFF_P0 + KT_ * N] = pack_k_tiles(P0, N)
    ident = np.eye(128, dtype=np.float32)
    blob[:, OFF_I:OFF_I + M] = ident
    blob[:, OFF_2I:OFF_2I + M] = r32r_rne(2.0 * ident)
    blob[:, OFF_X0:OFF_X0 + M] = r32r_rne(25.0 * ident)
    blob[:, OFF_RD:OFF_RD + M] = r32r_rne(R_COST * ident)
    qrow = np.zeros((128, KT_ * N), np.float32)
    for i in range(KT_):
        qrow[:, i * N + i * 128: i * N + (i + 1) * 128] = r32r_rne(Q_COST * ident)
    blob[:, OFF_QR:OFF_QR + KT_ * N] = qrow
    return blob


_CACHE = {}


def build(steps=STEPS, dump=False):
    nc = bacc.Bacc(trn_type="TRN2", target_bir_lowering=False)
    cb_d = nc.dram_tensor("cblob", [128, CBLOB], f32r, kind="ExternalInput")
    obs_d = nc.dram_tensor("obs", [SHARD, N], f32r, kind="ExternalInput")
    u0_d = (nc.dram_tensor("u0", [SHARD, M], f32, kind="ExternalOutput")
            if not dump else None)
    dump_d = (nc.dram_tensor("dump", [128, 6400], f32, kind="ExternalOutput")
              if dump else None)
    # [OGROUPS][128, OG_CH, 512] DRAM views
    obs_v = obs_d.ap().rearrange("(g c p) n -> g p c n", p=128, c=OG_CH)

    with tile.TileContext(nc) as tc:
        with tc.tile_pool(name="const", bufs=1) as cpool, \
             tc.tile_pool(name="obsp", bufs=1) as opool, \
             tc.tile_pool(name="stg", bufs=2) as spool, \
             tc.tile_pool(name="work", bufs=2) as wpool, \
             tc.tile_pool(name="pp", bufs=2) as ppool, \
             tc.tile_pool(name="big", bufs=4, space="PSUM") as psb, \
             tc.tile_pool(name="small", bufs=3, space="PSUM") as pss, \
             tc.tile_pool(name="nwt", bufs=1, space="PSUM") as psn:

            cb = cpool.tile([128, CBLOB], f32r, name="cb")
            nc.sync.dma_start(cb[:], cb_d.ap())
            A_s = cb[:, OFF_A:OFF_A + KT_ * N].rearrange("p (k n) -> p k n", k=KT_)
            B_s = cb[:, OFF_B:OFF_B + KT_ * M].rearrange("p (k n) -> p k n", k=KT_)
            P0_s = cb[:, OFF_P0:OFF_P0 + KT_ * N].rearrange("p (k n) -> p k n", k=KT_)
            I_s = cb[:, OFF_I:OFF_I + M]
            twoI_s = cb[:, OFF_2I:OFF_2I + M]
            X0_s = cb[:, OFF_X0:OFF_X0 + M]
            Rd_s = cb[:, OFF_RD:OFF_RD + M]
            QR_s = cb[:, OFF_QR:OFF_QR + KT_ * N].rearrange("p (k n) -> p k n", k=KT_)

            # transposed obs, filled incrementally: obsT[j][p, c*128+q] = obs[c*128+q, j*128+p]
            obsT = ([opool.tile([128, SHARD], f32r, name=f"obsT{j}")
                     for j in range(KT_)] if not dump else None)

            # obs transpose work queue: one item = one staged group's 16 transposes
            state = {"g": 0, "stage": None, "pos": 0}

            def emit_obs_transposes(budget):
                for _ in range(budget):
                    if state["g"] >= OGROUPS:
                        return
                    if state["stage"] is None:
                        stg = spool.tile([128, OG_CH, N], f32r, name="ostg",
                                         tag="ostg")
                        nc.sync.dma_start(stg[:], obs_v[state["g"]])
                        state["stage"] = stg
                        state["pos"] = 0
                    stg = state["stage"]
                    ci, j = divmod(state["pos"], KT_)
                    c = state["g"] * OG_CH + ci
                    tps = pss.tile([128, 128], f32r, name="otp", tag="sm")
                    nc.tensor.transpose(tps[:], stg[:, ci, j * 128:(j + 1) * 128],
                                        I_s)
                    nc.scalar.copy(obsT[j][:, c * 128:(c + 1) * 128], tps[:])
                    state["pos"] += 1
                    if state["pos"] == OG_CH * KT_:
                        state["g"] += 1
                        state["stage"] = None

            P_cur = [P0_s[:, k, :] for k in range(KT_)]
            X_cur = X0_s

            for t in range(steps):
                # Late odd steps: S (and X) barely change -- skip the whole
                # W/WT/S/Newton path and reuse the previous step's X.
                # t=48 must refresh S for the final K0 refinement.
                do_S = not (21 <= t <= 47 and t % 2 == 1)
                if do_S:
                    # W = B'P  [128, 512]
                    w_ps = psb.tile([128, N], f32, name="w", tag="big")
                    for k in range(KT_):
                        nc.tensor.matmul(w_ps[:], B_s[:, k, :], P_cur[k],
                                         start=(k == 0), stop=(k == KT_ - 1))
                    W = wpool.tile([128, N], f32r, name="W", tag="W")
                    nc.vector.tensor_copy(W[:], w_ps[:])

                # G = P A  (m-tile i: sum_k P_k[:, iblk].T @ A_k) -- emitted
                # early: depends only on P, keeps PE dense during the
                # WT/S/Newton dependency chain.
                G = [None] * KT_
                for i in range(KT_):
                    g_ps = psb.tile([128, N], f32, name=f"g{i}", tag="big")
                    for k in range(KT_):
                        nc.tensor.matmul(g_ps[:], P_cur[k][:, i * 128:(i + 1) * 128],
                                         A_s[:, k, :],
                                         start=(k == 0), stop=(k == KT_ - 1))
                    Gi = ppool.tile([128, N], f32r, name=f"G{i}", tag=f"G{i}")
                    nc.scalar.copy(Gi[:], g_ps[:])
                    G[i] = Gi

                if do_S:
                    # WT = W.T (= P B), 4 PE transposes
                    WT = wpool.tile([128, KT_, M], f32r, name="WT", tag="WT")
                    for j in range(KT_):
                        tps = pss.tile([128, 128], f32r, name="wtp", tag="sm")
                        nc.tensor.transpose(tps[:], W[:, j * 128:(j + 1) * 128], I_s)
                        nc.vector.tensor_copy(WT[:, j, :], tps[:])

                    # S = R + WT'B   [128,128]
                    s_ps = pss.tile([128, M], f32, name="s", tag="sm")
                    for k in range(KT_):
                        nc.tensor.matmul(s_ps[:], WT[:, k, :], B_s[:, k, :],
                                         start=(k == 0), stop=(k == KT_ - 1))
                    S = wpool.tile([128, M], f32r, name="S", tag="S")
                    nc.vector.tensor_add(S[:], Rd_s.bitcast(f32), s_ps[:])

                    # Newton-Schulz updates of X ~= S^-1.
                    # X' = (X'U + U'X)/2 : symmetric by construction. A plain
                    # X' = X.T @ U (lhsT=X) doubles the antisymmetric error each
                    # iteration and diverges after ~6 iterations on hardware.
                    for it in range(newton_iters(t)):
                        t_ps = pss.tile([128, M], f32, name="nt", tag="sm")
                        nc.tensor.matmul(t_ps[:], S[:], X_cur, start=True, stop=True)
                        U = wpool.tile([128, M], f32r, name="U", tag="U")
                        nc.vector.tensor_sub(U[:], twoI_s.bitcast(f32), t_ps[:])
                        x_ps = psn.tile([128, M], f32, name="nx", tag="nx")
                        nc.tensor.matmul(x_ps[:], X_cur, U[:], start=True, stop=False)
                        nc.tensor.matmul(x_ps[:], U[:], X_cur, start=False, stop=True)
                        Xn = wpool.tile([128, M], f32r, name="X", tag="X")
                        nc.vector.tensor_scalar_mul(Xn[:], x_ps[:], 0.5)
                        X_cur = Xn[:]

                # Y = B'G == B'PA  [128, 512] (from G: shorter chain than WT'A)
                y_ps = psb.tile([128, N], f32, name="y", tag="big")
                for k in range(KT_):
                    nc.tensor.matmul(y_ps[:], B_s[:, k, :], G[k][:],
                                     start=(k == 0), stop=(k == KT_ - 1))
                Y = wpool.tile([128, N], f32r, name="Y", tag="Y")
                nc.vector.tensor_copy(Y[:], y_ps[:])

                # XN = -X ; T1 = XN @ Y
                XN = wpool.tile([128, M], f32r, name="XN", tag="XN")
                nc.vector.tensor_scalar_mul(XN[:], X_cur, -1.0)
                t1_ps = psb.tile([128, N], f32, name="t1", tag="big")
                nc.tensor.matmul(t1_ps[:], XN[:], Y[:], start=True, stop=True)
                T1 = wpool.tile([128, N], f32r, name="T1", tag="T1")
                nc.scalar.copy(T1[:], t1_ps[:])

                # P'_i = Qrow_i + sum_k A_k[:, iblk].T @ G_k + Y[:, iblk].T @ T1
                P_new = [None] * KT_
                for i in range(KT_):
                    # P' is symmetric: for i=1,2 compute only block-cols
                    # j >= i and mirror the lower blocks from earlier tiles
                    # via PE transposes. i=3 stays full width: N=128 fp32r
                    # pays 4 cyc/row, so the narrow matmul saves nothing.
                    lo = i * 128 if i in (1, 2) else 0
                    p_ps = psb.tile([128, N], f32, name=f"p{i}", tag="big")
                    for k in range(KT_):
                        nc.tensor.matmul(p_ps[:, lo:N],
                                         A_s[:, k, i * 128:(i + 1) * 128],
                                         G[k][:, lo:N], start=(k == 0), stop=False)
                    nc.tensor.matmul(p_ps[:, lo:N], Y[:, i * 128:(i + 1) * 128],
                                     T1[:, lo:N], start=False, stop=True)
                    Pi = ppool.tile([128, N], f32r, name=f"P{i}", tag=f"P{i}")
                    nc.vector.tensor_add(Pi[:, lo:N],
                                         QR_s[:, i, lo:N].bitcast(f32),
                                         p_ps[:, lo:N])
                    for j in range(i if i in (1, 2) else 0):
                        mps = pss.tile([128, 128], f32r, name="mtp", tag="sm")
                        nc.tensor.transpose(
                            mps[:], P_new[j][:, i * 128:(i + 1) * 128], I_s)
                        eng = nc.vector if (i + j) % 2 == 0 else nc.scalar
                        if eng is nc.vector:
                            nc.vector.tensor_copy(Pi[:, j * 128:(j + 1) * 128], mps[:])
                        else:
                            nc.scalar.copy(Pi[:, j * 128:(j + 1) * 128], mps[:])
                    P_new[i] = Pi

                P_cur = [P_new[i][:] for i in range(KT_)]
                if not dump:
                    emit_obs_transposes(3)
                if dump and t == steps - 1:
                    dmp = opool.tile([128, 6400], f32, name="dump_sb")
                    nc.vector.tensor_copy(dmp[:, 0:512], W[:].bitcast(f32))
                    nc.vector.tensor_copy(dmp[:, 512:1024], WT[:].rearrange("p k n -> p (k n)").bitcast(f32))
                    nc.vector.tensor_copy(dmp[:, 1024:1152], S[:].bitcast(f32))
                    nc.vector.tensor_copy(dmp[:, 1152:1664], Y[:].bitcast(f32))
                    nc.vector.tensor_copy(dmp[:, 1664:1792], X_cur.bitcast(f32))
                    nc.vector.tensor_copy(dmp[:, 1792:2304], T1[:].bitcast(f32))
                    for i in range(KT_):
                        nc.vector.tensor_copy(dmp[:, 2304 + i * 512:2816 + i * 512], G[i][:].bitcast(f32))
                        nc.vector.tensor_copy(dmp[:, 4352 + i * 512:4864 + i * 512], P_new[i][:].bitcast(f32))
                    nc.sync.dma_start(dump_d.ap(), dmp[:])

            # drain remaining obs transposes
            if not dump:
                emit_obs_transposes(OGROUPS * OG_CH * KT_)

            if not dump:
                # K0 = X Y with one refinement: K0' = K0 + X (Y - S K0)
                k0_ps = psb.tile([128, N], f32, name="k0", tag="big")
                nc.tensor.matmul(k0_ps[:], X_cur, Y[:], start=True, stop=False)
                K0a = wpool.tile([128, N], f32r, name="K0a", tag="K0a")
                nc.vector.tensor_copy(K0a[:], k0_ps[:])
                sk_ps = psb.tile([128, N], f32, name="sk", tag="big")
                nc.tensor.matmul(sk_ps[:], S[:], K0a[:], start=True, stop=True)
                E = wpool.tile([128, N], f32r, name="E", tag="E")
                nc.vector.tensor_sub(E[:], Y[:].bitcast(f32), sk_ps[:])
                nc.tensor.matmul(k0_ps[:], X_cur, E[:], start=False, stop=True)
                K0 = wpool.tile([128, N], f32r, name="K0", tag="K0")
                nc.vector.tensor_copy(K0[:], k0_ps[:])

                # K0T (= -K0.T) via PE transposes, sign folded into the copy
                K0T = wpool.tile([128, KT_, M], f32r, name="K0T", tag="K0T")
                for j in range(KT_):
                    tps = pss.tile([128, 128], f32r, name="ktp", tag="sm")
                    nc.tensor.transpose(tps[:], K0[:, j * 128:(j + 1) * 128], I_s)
                    nc.vector.tensor_scalar_mul(K0T[:, j, :], tps[:].bitcast(f32), -1.0)

                # u0T = -K0 @ obs.T computed per 512-col group, then each
                # [128,128] block is PE-transposed back to natural u0 layout
                u0_sb = opool.tile([128, CHUNKS, M], f32, name="u0")
                for g in range(SHARD // N):
                    u_ps = psb.tile([128, N], f32, name=f"u{g}", tag="big")
                    for k in range(KT_):
                        nc.tensor.matmul(u_ps[:], K0T[:, k, :],
                                         obsT[k][:, g * N:(g + 1) * N],
                                         start=(k == 0), stop=(k == KT_ - 1))
                    ut = wpool.tile([128, N], f32, name="UT", tag="UT")
                    nc.scalar.copy(ut[:], u_ps[:])
                    for q in range(KT_):
                        c = g * KT_ + q
                        tps2 = pss.tile([128, 128], f32, name="utp", tag="sm")
                        nc.tensor.transpose(tps2[:], ut[:, q * 128:(q + 1) * 128],
                                            I_s.bitcast(f32))
                        nc.vector.tensor_copy(u0_sb[:, c, :], tps2[:])
                nc.sync.dma_start(u0_d.ap().rearrange("(c p) m -> p c m", p=128),
                                  u0_sb[:])
    nc.finalize()
    return nc


def kernel(obs, A, B):
    obs = np.ascontiguousarray(obs, np.float32)
    cblob = build_const_blob(np.asarray(A, np.float32),
                             np.asarray(B, np.float32))
    if "nc" not in _CACHE:
        _CACHE["nc"] = build()
    nc = _CACHE["nc"]
    in_maps = [{"cblob": cblob, "obs": obs[c * SHARD:(c + 1) * SHARD]}
               for c in range(NCORES)]
    res = bass_utils.run_bass_kernel_spmd(nc, in_maps, core_ids=list(range(NCORES)))
    return np.concatenate([r["u0"] for r in res.results], axis=0)



# revision 5
# speedup vs baseline: 1.1546x; 1.1546x over previous
"""Trainium2 Bass kernel for nn_CvxMPC: finite-horizon LQR gain (Riccati
recursion) + batch control computation  u0 = -obs @ K0.T.

Sharding: obs is split along batch across 8 cores (data parallel); A, B and
the entire Riccati recursion are replicated on every core (no collectives).

Device algorithm per core (wide matmuls fp32r, narrow (128-wide) matmuls
bf16 — the PE pays 4 cyc/row for fp32r below 256-wide but 1 for bf16;
fp32 PSUM accumulate; PE computes lhsT.T @ rhs):
    P = Q
    repeat 49x:
        W  = B'P            (lhsT = B; t=0: skipped, WT = 0.01 B)
        WT = W.T            (PE transpose, bf16)  == P B
        S  = R + WT'B       (bf16)
        X ~= S^-1           (Newton-Schulz, bf16, warm started across steps)
        G  = P'A == PA      (P symmetric; t=0: G = 0.01 A via DVE)
        Y  = B'G            == B'PA
        T1 = (-X) Y
        P' = Q + A'G + Y'T1 (PSUM accumulation; == Q + A'PA - Y'XY)
    K0 = X Y (+ one Newton refinement);  u0.T = -K0 @ obs.T
obs.T is built with PE transposes interleaved into the Riccati stream.
"""
import numpy as np
import concourse.bacc as bacc
import concourse.mybir as mybir
import concourse.tile as tile
from concourse import bass_utils

f32 = mybir.dt.float32
f32r = mybir.dt.float32r
bf16 = mybir.dt.bfloat16

N = 512          # state dim
M = 128          # control dim
STEPS = 49       # HORIZON - 1
Q_COST = 0.01
R_COST = 0.01
BATCH = 32768
NCORES = 8
SHARD = BATCH // NCORES          # 4096 rows per core
CHUNKS = SHARD // 128            # 32 [128,512] obs row-chunks per core
KT_ = N // 128                   # 4 k-tiles
OGROUPS = 8                      # obs DMA groups (4 chunks each)
OG_CH = CHUNKS // OGROUPS        # 4

X0_SCALE = 44.0  # ~2/(lmin+lmax) of S_0 = 0.01(I + B'B); NS contraction ~0.45


def newton_iters(t):
    """Newton-Schulz iteration schedule (prototype-validated with margin)."""
    if t == 0:
        return 7
    if t < 4:
        return 4
    if t < 10:
        return 3
    if t < 20:
        return 2
    return 1


def r32r_rne(x):
    """Round fp32 -> fp32r (11-bit mantissa), round-to-nearest-even.
    Matches the DVE f32->f32r cast measured on hardware."""
    u = np.ascontiguousarray(x, np.float32).view(np.uint32).copy()
    bias = np.uint32(0x7FF) + ((u >> np.uint32(12)) & np.uint32(1))
    u = (u + bias) & np.uint32(0xFFFFF000)
    return u.view(np.float32)


# ---- constant blob layout (per-partition f32 elements) ----
# ordered so one small leading DMA unblocks the t=0 Newton chain
OFF_B = 0                      # B  [4 x 128] k-partition tiles, fp32r
OFF_I = OFF_B + KT_ * M        # identity [128]
OFF_2I = OFF_I + M             # 2*I [128]
OFF_X0 = OFF_2I + M            # X0 = X0_SCALE*I [128]
OFF_RD = OFF_X0 + M            # R diag = 0.01*I [128]
SMALL = OFF_RD + M             # end of the small leading section
OFF_A = SMALL                  # A  [4 x 512]
OFF_QR = OFF_A + KT_ * N       # Q row tiles [4 x 512] (0.01*I block at i)
CBLOB = OFF_QR + KT_ * N


def pack_k_tiles(x, width):
    """[512, width] -> [128, 4*width] with k-partition tiling."""
    return np.ascontiguousarray(
        x.reshape(KT_, 128, width).transpose(1, 0, 2).reshape(128, KT_ * width))


def build_const_blob(A, B):
    blob = np.zeros((128, CBLOB), np.float32)
    blob[:, OFF_A:OFF_A + KT_ * N] = pack_k_tiles(r32r_rne(A), N)
    blob[:, OFF_B:OFF_B + KT_ * M] = pack_k_tiles(r32r_rne(B), M)
    ident = np.eye(128, dtype=np.float32)
    blob[:, OFF_I:OFF_I + M] = ident
    blob[:, OFF_2I:OFF_2I + M] = r32r_rne(2.0 * ident)
    blob[:, OFF_X0:OFF_X0 + M] = r32r_rne(X0_SCALE * ident)
    blob[:, OFF_RD:OFF_RD + M] = r32r_rne(R_COST * ident)
    qrow = np.zeros((128, KT_ * N), np.float32)
    for i in range(KT_):
        qrow[:, i * N + i * 128: i * N + (i + 1) * 128] = r32r_rne(Q_COST * ident)
    blob[:, OFF_QR:OFF_QR + KT_ * N] = qrow
    return blob


_CACHE = {}


def build(steps=STEPS):
    nc = bacc.Bacc(trn_type="TRN2", target_bir_lowering=False)
    cb_d = nc.dram_tensor("cblob", [128, CBLOB], f32r, kind="ExternalInput")
    obs_d = nc.dram_tensor("obs", [SHARD, N], f32r, kind="ExternalInput")
    u0_d = nc.dram_tensor("u0", [SHARD, M], f32, kind="ExternalOutput")
    # [OGROUPS][128, OG_CH, 512] DRAM views
    obs_v = obs_d.ap().rearrange("(g c p) n -> g p c n", p=128, c=OG_CH)
    u0_v = u0_d.ap().rearrange("(g c p) m -> g p c m", p=128, c=OG_CH)

    with tile.TileContext(nc) as tc:
        with tc.tile_pool(name="const", bufs=1) as cpool, \
             tc.tile_pool(name="obsp", bufs=1) as opool, \
             tc.tile_pool(name="stg", bufs=2) as spool, \
             tc.tile_pool(name="work", bufs=2) as wpool, \
             tc.tile_pool(name="pp", bufs=2) as ppool, \
             tc.tile_pool(name="big", bufs=4, space="PSUM") as psb, \
             tc.tile_pool(name="small", bufs=3, space="PSUM") as pss, \
             tc.tile_pool(name="nwt", bufs=1, space="PSUM") as psn:

            cb = cpool.tile([128, CBLOB], f32r, name="cb")
            nc.sync.dma_start(cb[:, 0:SMALL], cb_d.ap()[:, 0:SMALL])
            nc.sync.dma_start(cb[:, SMALL:CBLOB], cb_d.ap()[:, SMALL:CBLOB])
            A_s = cb[:, OFF_A:OFF_A + KT_ * N].rearrange("p (k n) -> p k n", k=KT_)
            B_s = cb[:, OFF_B:OFF_B + KT_ * M].rearrange("p (k n) -> p k n", k=KT_)
            I_s = cb[:, OFF_I:OFF_I + M]
            twoI_s = cb[:, OFF_2I:OFF_2I + M]
            X0_s = cb[:, OFF_X0:OFF_X0 + M]
            Rd_s = cb[:, OFF_RD:OFF_RD + M]
            QR_s = cb[:, OFF_QR:OFF_QR + KT_ * N].rearrange("p (k n) -> p k n", k=KT_)

            # bf16 copies of the constants the narrow-matmul path needs
            B16 = cpool.tile([128, KT_, M], bf16, name="B16")
            nc.vector.tensor_copy(B16[:].rearrange("p k n -> p (k n)"),
                                  cb[:, OFF_B:OFF_B + KT_ * M].bitcast(f32))
            I16 = cpool.tile([128, M], bf16, name="I16")
            nc.vector.tensor_copy(I16[:], I_s.bitcast(f32))

            # transposed obs, filled incrementally
            obsT = [opool.tile([128, SHARD], f32r, name=f"obsT{j}")
                    for j in range(KT_)]

            # obs transpose work queue: one item = one staged group's 16 transposes
            state = {"g": 0, "stage": None, "pos": 0}

            def emit_obs_transposes(budget):
                for _ in range(budget):
                    if state["g"] >= OGROUPS:
                        return
                    if state["stage"] is None:
                        stg = spool.tile([128, OG_CH, N], f32r, name="ostg",
                                         tag="ostg")
                        nc.sync.dma_start(stg[:], obs_v[state["g"]])
                        state["stage"] = stg
                        state["pos"] = 0
                    stg = state["stage"]
                    ci, j = divmod(state["pos"], KT_)
                    c = state["g"] * OG_CH + ci
                    tps = pss.tile([128, 128], f32r, name="otp", tag="sm")
                    nc.tensor.transpose(tps[:], stg[:, ci, j * 128:(j + 1) * 128],
                                        I_s)
                    nc.scalar.copy(obsT[j][:, c * 128:(c + 1) * 128], tps[:])
                    state["pos"] += 1
                    if state["pos"] == OG_CH * KT_:
                        state["g"] += 1
                        state["stage"] = None

            P_cur = None          # [4][128, 512] f32r tiles (None at t=0: P=Q)
            X16 = cpool.tile([128, M], bf16, name="X16w")
            nc.vector.tensor_copy(X16[:], X0_s.bitcast(f32))
            XN = None             # -X in f32r (persistent across skip steps)
            S16 = None
            x_ps_last = None

            for t in range(steps):
                # Late odd steps: S (and X) barely change -- skip the whole
                # W/WT/S/Newton path and reuse the previous step's X.
                # t=48 must refresh S for the final K0 refinement.
                do_S = not (21 <= t <= 47 and t % 2 == 1)
                last = t == steps - 1
                if do_S:
                    WT16 = wpool.tile([128, KT_, M], bf16, name="WT", tag="WT")
                    if t == 0:
                        # P = Q: WT = PB = 0.01 B
                        nc.vector.tensor_scalar_mul(
                            WT16[:].rearrange("p k n -> p (k n)"),
                            B16[:].rearrange("p k n -> p (k n)"), Q_COST)
                    else:
                        # W = B'P  [128, 512]
                        w_ps = psb.tile([128, N], f32, name="w", tag="big")
                        for k in range(KT_):
                            nc.tensor.matmul(w_ps[:], B_s[:, k, :], P_cur[k],
                                             start=(k == 0), stop=(k == KT_ - 1))
                        W16 = wpool.tile([128, N], bf16, name="W", tag="W")
                        nc.vector.tensor_copy(W16[:], w_ps[:])
                        # WT = W.T (= P B), 4 PE transposes (bf16)
                        for j in range(KT_):
                            tps = pss.tile([128, 128], bf16, name="wtp", tag="sm")
                            nc.tensor.transpose(tps[:], W16[:, j * 128:(j + 1) * 128],
                                                I16[:])
                            nc.vector.tensor_copy(WT16[:, j, :], tps[:])

                    # S = R + WT'B   [128,128] (bf16 matmuls)
                    s_ps = pss.tile([128, M], f32, name="s", tag="sm")
                    for k in range(KT_):
                        nc.tensor.matmul(s_ps[:], WT16[:, k, :], B16[:, k, :],
                                         start=(k == 0), stop=(k == KT_ - 1))
                    S16 = wpool.tile([128, M], bf16, name="S", tag="S")
                    nc.vector.tensor_add(S16[:], Rd_s.bitcast(f32), s_ps[:])
                    if last:
                        Sr = wpool.tile([128, M], f32r, name="Sr", tag="Sr")
                        nc.vector.tensor_add(Sr[:], Rd_s.bitcast(f32), s_ps[:])

                # G = P A  (m-tile i) -- emitted early: depends only on P,
                # keeps PE dense during the WT/S/Newton dependency chain.
                G = [None] * KT_
                if t == 0:
                    # P = Q: G = 0.01 A (DVE scale-copy, no matmuls)
                    for i in range(KT_):
                        Gi = ppool.tile([128, N], f32r, name=f"G{i}", tag=f"G{i}")
                        nc.vector.tensor_scalar_mul(Gi[:], A_s[:, i, :], Q_COST)
                        G[i] = Gi
                else:
                    for i in range(KT_):
                        g_ps = psb.tile([128, N], f32, name=f"g{i}", tag="big")
                        for k in range(KT_):
                            nc.tensor.matmul(g_ps[:],
                                             P_cur[k][:, i * 128:(i + 1) * 128],
                                             A_s[:, k, :],
                                             start=(k == 0), stop=(k == KT_ - 1))
                        Gi = ppool.tile([128, N], f32r, name=f"G{i}", tag=f"G{i}")
                        nc.scalar.copy(Gi[:], g_ps[:])
                        G[i] = Gi

                if do_S:
                    # Newton-Schulz updates of X ~= S^-1 (bf16 narrow matmuls).
                    # X' = (X'U + U'X)/2 : symmetric by construction (a plain
                    # X' = X U doubles the antisymmetric error and diverges).
                    for it in range(newton_iters(t)):
                        t_ps = pss.tile([128, M], f32, name="nt", tag="sm")
                        nc.tensor.matmul(t_ps[:], S16[:], X16[:], start=True,
                                         stop=True)
                        U16 = wpool.tile([128, M], bf16, name="U", tag="U")
                        nc.vector.tensor_sub(U16[:], twoI_s.bitcast(f32), t_ps[:])
                        x_ps = psn.tile([128, M], f32, name="nx", tag="nx")
                        nc.tensor.matmul(x_ps[:], X16[:], U16[:], start=True,
                                         stop=False)
                        nc.tensor.matmul(x_ps[:], U16[:], X16[:], start=False,
                                         stop=True)
                        X16 = wpool.tile([128, M], bf16, name="X", tag="X")
                        nc.vector.tensor_scalar_mul(X16[:], x_ps[:], 0.5)
                        x_ps_last = x_ps
                    # -X in f32r for the wide T1/K0 matmuls
                    XN = wpool.tile([128, M], f32r, name="XN", tag="XN")
                    nc.vector.tensor_scalar_mul(XN[:], x_ps_last[:], -0.5)

                # Y = B'G == B'PA  [128, 512]
                y_ps = psb.tile([128, N], f32, name="y", tag="big")
                for k in range(KT_):
                    nc.tensor.matmul(y_ps[:], B_s[:, k, :], G[k][:],
                                     start=(k == 0), stop=(k == KT_ - 1))
                Y = wpool.tile([128, N], f32r, name="Y", tag="Y")
                nc.vector.tensor_copy(Y[:], y_ps[:])

                # T1 = (-X) @ Y
                t1_ps = psb.tile([128, N], f32, name="t1", tag="big")
                nc.tensor.matmul(t1_ps[:], XN[:], Y[:], start=True, stop=True)
                T1 = wpool.tile([128, N], f32r, name="T1", tag="T1")
                nc.scalar.copy(T1[:], t1_ps[:])

                # P'_i = Qrow_i + sum_k A_k[:, iblk].T @ G_k + Y[:, iblk].T @ T1
                P_new = [None] * KT_
                for i in range(KT_):
                    # P' is symmetric: for i=1,2 compute only block-cols
                    # j >= i and mirror the lower blocks from earlier tiles
                    # via PE transposes. i=3 stays full width: N=128 fp32r
                    # pays 4 cyc/row, so the narrow matmul saves nothing.
                    lo = i * 128 if i in (1, 2) else 0
                    p_ps = psb.tile([128, N], f32, name=f"p{i}", tag="big")
                    for k in range(KT_):
                        nc.tensor.matmul(p_ps[:, lo:N],
                                         A_s[:, k, i * 128:(i + 1) * 128],
                                         G[k][:, lo:N], start=(k == 0), stop=False)
                    nc.tensor.matmul(p_ps[:, lo:N], Y[:, i * 128:(i + 1) * 128],
                                     T1[:, lo:N], start=False, stop=True)
                    Pi = ppool.tile([128, N], f32r, name=f"P{i}", tag=f"P{i}")
                    nc.vector.tensor_add(Pi[:, lo:N],
                                         QR_s[:, i, lo:N].bitcast(f32),
                                         p_ps[:, lo:N])
                    for j in range(i if i in (1, 2) else 0):
                        mps = pss.tile([128, 128], f32r, name="mtp", tag="sm")
                        nc.tensor.transpose(
                            mps[:], P_new[j][:, i * 128:(i + 1) * 128], I_s)
                        eng = nc.vector if (i + j) % 2 == 0 else nc.scalar
                        if eng is nc.vector:
                            nc.vector.tensor_copy(Pi[:, j * 128:(j + 1) * 128], mps[:])
                        else:
                            nc.scalar.copy(Pi[:, j * 128:(j + 1) * 128], mps[:])
                    P_new[i] = Pi

                P_cur = [P_new[i][:] for i in range(KT_)]
                emit_obs_transposes(3)

            # drain remaining obs transposes
            emit_obs_transposes(OGROUPS * OG_CH * KT_)

            # K0 = X Y with one refinement: K0' = K0 + X (Y - S K0)
            # (-X) available as XN; signs: K0a = -(XN Y);  use K0n = XN Y = -K0
            k0_ps = psb.tile([128, N], f32, name="k0", tag="big")
            nc.tensor.matmul(k0_ps[:], XN[:], Y[:], start=True, stop=False)
            K0a = wpool.tile([128, N], f32r, name="K0a", tag="K0a")
            nc.vector.tensor_scalar_mul(K0a[:], k0_ps[:], -1.0)  # = X Y
            sk_ps = psb.tile([128, N], f32, name="sk", tag="big")
            nc.tensor.matmul(sk_ps[:], Sr[:], K0a[:], start=True, stop=True)
            E = wpool.tile([128, N], f32r, name="E", tag="E")
            nc.vector.tensor_sub(E[:], Y[:].bitcast(f32), sk_ps[:])
            # k0_ps currently holds -(X Y); add -(X E): total -(K0 + X E) = -K0'
            nc.tensor.matmul(k0_ps[:], XN[:], E[:], start=False, stop=True)
            K0n = wpool.tile([128, N], f32r, name="K0n", tag="K0n")
            nc.vector.tensor_copy(K0n[:], k0_ps[:])   # = -K0'

            # K0T = (-K0).T via PE transposes (K0n already = -K0)
            K0T = wpool.tile([128, KT_, M], f32r, name="K0T", tag="K0T")
            for j in range(KT_):
                tps = pss.tile([128, 128], f32r, name="ktp", tag="sm")
                nc.tensor.transpose(tps[:], K0n[:, j * 128:(j + 1) * 128], I_s)
                nc.vector.tensor_copy(K0T[:, j, :], tps[:])

            # u0T = -K0 @ obs.T computed per 512-col group, then each
            # [128,128] block is PE-transposed back to natural u0 layout;
            # each group's result DMAs out immediately (overlaps the next).
            for g in range(SHARD // N):
                u_ps = psb.tile([128, N], f32, name=f"u{g}", tag="big")
                for k in range(KT_):
                    nc.tensor.matmul(u_ps[:], K0T[:, k, :],
                                     obsT[k][:, g * N:(g + 1) * N],
                                     start=(k == 0), stop=(k == KT_ - 1))
                ut = wpool.tile([128, N], f32r, name="UT", tag="UT")
                nc.scalar.copy(ut[:], u_ps[:])
                u0g = spool.tile([128, OG_CH, M], f32, name="u0g", tag="u0g")
                for q in range(KT_):
                    tps2 = pss.tile([128, 128], f32r, name="utp", tag="sm")
                    nc.tensor.transpose(tps2[:], ut[:, q * 128:(q + 1) * 128],
                                        I_s)
                    nc.vector.tensor_copy(u0g[:, q, :], tps2[:])
                nc.sync.dma_start(u0_v[g], u0g[:])
    nc.finalize()
    return nc


def kernel(obs, A, B):
    obs = np.ascontiguousarray(obs, np.float32)
    cblob = build_const_blob(np.asarray(A, np.float32),
                             np.asarray(B, np.float32))
    if "nc" not in _CACHE:
        _CACHE["nc"] = build()
    nc = _CACHE["nc"]
    in_maps = [{"cblob": cblob, "obs": obs[c * SHARD:(c + 1) * SHARD]}
               for c in range(NCORES)]
    res = bass_utils.run_bass_kernel_spmd(nc, in_maps, core_ids=list(range(NCORES)))
    return np.concatenate([r["u0"] for r in res.results], axis=0)


# revision 9
# speedup vs baseline: 1.6668x; 1.4437x over previous
"""Trainium2 Bass kernel for nn_CvxMPC: finite-horizon LQR gain (Riccati
recursion) + batch control computation  u0 = -obs @ K0.T.

Sharding: obs is split along batch across 8 cores (data parallel); A, B and
the entire Riccati recursion are replicated on every core (no collectives).

The 49-step Riccati recursion is run as 24 *composite double steps* (exact
2-step composition -- two plain steps fused into one step with constants
A2 = A^2, Bt = [AB, B], Qt = Q + A'QA, Rt = blkdiag(R + B'QB, R),
C = [A'QB, 0], all host-precomputed P-independent constant folding), then
one single-step gain extraction on P_48:

    per double step (P <- Qt + A2'PA2 - Yt' St^-1 Yt,  St = Rt + Bt'P Bt,
                     Yt = Bt'P A2 + C'):
        W~ = Bt'P, WT = W~.T (PE transposes) == P Bt
        St blocks S11(+R11), S12, S21, S22(+R) from Bt' WT
        X ~= S22^-1, D ~= (S11 - S12 X S21)^-1 (warm-started bf16
            Newton-Schulz; the 2x2 block inverse in negated f32r tiles:
            O11n=-D, O12n=+D S12 X, O21n=+(X S21)D, O22n=-(X + V D V'))
        G = P A2;  Yt = Bt'G + C'
        z = -St^-1 Yt via the O blocks;  P' = Qt + A2'G + Yt'z
    finally: S = R + B'P48 B, X ~= S^-1, Y = B'P48 A, K0 = X Y (+1 Newton
    refinement), u0.T = -K0 @ obs.T.

Wide matmuls are fp32r (1 cyc/row at >=256 out width); narrow 128-wide
matmuls are bf16 (fp32r pays 4 cyc/row there). fp32 PSUM accumulate.
obs.T is built with PE transposes interleaved into the Riccati stream.
"""
import numpy as np
import concourse.bacc as bacc
import concourse.mybir as mybir
import concourse.tile as tile
from concourse import bass_utils

f32 = mybir.dt.float32
f32r = mybir.dt.float32r
bf16 = mybir.dt.bfloat16

N = 512          # state dim
M = 128          # control dim
DOUBLES = 24     # composite double steps: P_0=Q -> P_48
Q_COST = 0.01
R_COST = 0.01
BATCH = 32768
NCORES = 8
SHARD = BATCH // NCORES          # 4096 rows per core
CHUNKS = SHARD // 128            # 32 [128,512] obs row-chunks per core
KT_ = N // 128                   # 4 k-tiles
OGROUPS = 8                      # obs DMA groups (4 chunks each)
OG_CH = CHUNKS // OGROUPS        # 4

X0_SCALE = 44.0  # ~2/(lmin+lmax) of S22_0 = 0.01(I + B'B)
D0_SCALE = 26.0  # ~2/(lmin+lmax) of the d=0 Schur complement


def newton_iters(d):
    """Per-double-step Newton-Schulz iterations (prototype-validated)."""
    if d == 0:
        return 7
    return 2 if d < 3 else 1


def refresh(d):
    """Whether double step d refreshes W~/St/X/D/O (else reuse O blocks)."""
    return d < 8 or d % 2 == 0


def r32r_rne(x):
    """Round fp32 -> fp32r (11-bit mantissa), round-to-nearest-even."""
    u = np.ascontiguousarray(x, np.float32).view(np.uint32).copy()
    bias = np.uint32(0x7FF) + ((u >> np.uint32(12)) & np.uint32(1))
    u = (u + bias) & np.uint32(0xFFFFF000)
    return u.view(np.float32)


# ---- constant blob layout (per-partition f32 elements) ----
# small leading section unblocks the d=0 Newton chain immediately
OFF_BT = 0                         # Bt=[AB,B] [4 x 256] k-tiles
OFF_I = OFF_BT + KT_ * 2 * M       # identity [128]
OFF_2I = OFF_I + M                 # 2*I [128]
OFF_X0 = OFF_2I + M                # X0_SCALE*I [128]
OFF_D0 = OFF_X0 + M                # D0_SCALE*I [128]
OFF_RD = OFF_D0 + M                # R diag = 0.01*I [128]
OFF_R11 = OFF_RD + M               # R + B'QB  [128x128 dense]
SMALL = OFF_R11 + M
OFF_A2 = SMALL                     # A^2 [4 x 512] k-tiles
OFF_CT = OFF_A2 + KT_ * N          # C' top block = 0.01*B'A  [128 x 512]
OFF_QT = OFF_CT + N                # Qt row tiles [4 x 512]
OFF_A1 = OFF_QT + KT_ * N          # A [4 x 512] k-tiles (final step only)
CBLOB = OFF_A1 + KT_ * N


def pack_k_tiles(x, width):
    return np.ascontiguousarray(
        x.reshape(KT_, 128, width).transpose(1, 0, 2).reshape(128, KT_ * width))


def build_const_blob(A, B):
    A = np.asarray(A, np.float64)
    B = np.asarray(B, np.float64)
    Q = Q_COST * np.eye(N)
    A2 = A @ A
    Bt = np.concatenate([A @ B, B], axis=1)
    Qt = Q + A.T @ Q @ A
    R11 = R_COST * np.eye(M) + B.T @ Q @ B
    CT = Q_COST * (B.T @ A)            # C' top block [M, N]

    blob = np.zeros((128, CBLOB), np.float32)
    blob[:, OFF_BT:OFF_BT + KT_ * 2 * M] = pack_k_tiles(
        r32r_rne(Bt.astype(np.float32)), 2 * M)
    ident = np.eye(128, dtype=np.float32)
    blob[:, OFF_I:OFF_I + M] = ident
    blob[:, OFF_2I:OFF_2I + M] = r32r_rne(2.0 * ident)
    blob[:, OFF_X0:OFF_X0 + M] = r32r_rne(X0_SCALE * ident)
    blob[:, OFF_D0:OFF_D0 + M] = r32r_rne(D0_SCALE * ident)
    blob[:, OFF_RD:OFF_RD + M] = r32r_rne(R_COST * ident)
    blob[:, OFF_R11:OFF_R11 + M] = r32r_rne(R11.astype(np.float32))
    blob[:, OFF_A2:OFF_A2 + KT_ * N] = pack_k_tiles(
        r32r_rne(A2.astype(np.float32)), N)
    blob[:, OFF_CT:OFF_CT + N] = r32r_rne(CT.astype(np.float32))
    blob[:, OFF_QT:OFF_QT + KT_ * N] = pack_k_tiles(
        r32r_rne(Qt.astype(np.float32)), N)
    blob[:, OFF_A1:OFF_A1 + KT_ * N] = pack_k_tiles(
        r32r_rne(A.astype(np.float32)), N)
    return blob


_CACHE = {}


def build(doubles=DOUBLES):
    nc = bacc.Bacc(trn_type="TRN2", target_bir_lowering=False)
    cb_d = nc.dram_tensor("cblob", [128, CBLOB], f32r, kind="ExternalInput")
    obs_d = nc.dram_tensor("obs", [SHARD, N], f32r, kind="ExternalInput")
    u0_d = nc.dram_tensor("u0", [SHARD, M], f32, kind="ExternalOutput")
    obs_v = obs_d.ap().rearrange("(g c p) n -> g p c n", p=128, c=OG_CH)
    u0_v = u0_d.ap().rearrange("(g c p) m -> g p c m", p=128, c=OG_CH)

    with tile.TileContext(nc) as tc:
        with tc.tile_pool(name="const", bufs=1) as cpool, \
             tc.tile_pool(name="obsp", bufs=1) as opool, \
             tc.tile_pool(name="stg", bufs=2) as spool, \
             tc.tile_pool(name="work", bufs=2) as wpool, \
             tc.tile_pool(name="pp", bufs=2) as ppool, \
             tc.tile_pool(name="big", bufs=4, space="PSUM") as psb, \
             tc.tile_pool(name="small", bufs=3, space="PSUM") as pss, \
             tc.tile_pool(name="nwt", bufs=1, space="PSUM") as psn:

            cb = cpool.tile([128, CBLOB], f32r, name="cb")
            nc.sync.dma_start(cb[:, 0:SMALL], cb_d.ap()[:, 0:SMALL])
            nc.sync.dma_start(cb[:, SMALL:CBLOB], cb_d.ap()[:, SMALL:CBLOB])
            Bt_s = cb[:, OFF_BT:OFF_BT + KT_ * 2 * M].rearrange(
                "p (k n) -> p k n", k=KT_)
            I_s = cb[:, OFF_I:OFF_I + M]
            twoI_s = cb[:, OFF_2I:OFF_2I + M]
            X0_s = cb[:, OFF_X0:OFF_X0 + M]
            D0_s = cb[:, OFF_D0:OFF_D0 + M]
            Rd_s = cb[:, OFF_RD:OFF_RD + M]
            R11_s = cb[:, OFF_R11:OFF_R11 + M]
            A2_s = cb[:, OFF_A2:OFF_A2 + KT_ * N].rearrange(
                "p (k n) -> p k n", k=KT_)
            CT_s = cb[:, OFF_CT:OFF_CT + N]
            QT_s = cb[:, OFF_QT:OFF_QT + KT_ * N].rearrange(
                "p (k n) -> p k n", k=KT_)
            A1_s = cb[:, OFF_A1:OFF_A1 + KT_ * N].rearrange(
                "p (k n) -> p k n", k=KT_)

            # bf16 copies of narrow-matmul constants
            Bt16 = cpool.tile([128, KT_, 2 * M], bf16, name="Bt16")
            nc.vector.tensor_copy(Bt16[:].rearrange("p k n -> p (k n)"),
                                  cb[:, OFF_BT:OFF_BT + KT_ * 2 * M].bitcast(f32))
            I16 = cpool.tile([128, M], bf16, name="I16")
            nc.vector.tensor_copy(I16[:], I_s.bitcast(f32))

            obsT = [opool.tile([128, SHARD], f32r, name=f"obsT{j}")
                    for j in range(KT_)]
            state = {"g": 0, "stage": None, "pos": 0}

            def emit_obs_transposes(budget):
                for _ in range(budget):
                    if state["g"] >= OGROUPS:
                        return
                    if state["stage"] is None:
                        stg = spool.tile([128, OG_CH, N], f32r, name="ostg",
                                         tag="ostg")
                        nc.sync.dma_start(stg[:], obs_v[state["g"]])
                        state["stage"] = stg
                        state["pos"] = 0
                    stg = state["stage"]
                    ci, j = divmod(state["pos"], KT_)
                    c = state["g"] * OG_CH + ci
                    tps = pss.tile([128, 128], f32r, name="otp", tag="sm")
                    nc.tensor.transpose(tps[:], stg[:, ci, j * 128:(j + 1) * 128],
                                        I_s)
                    nc.scalar.copy(obsT[j][:, c * 128:(c + 1) * 128], tps[:])
                    state["pos"] += 1
                    if state["pos"] == OG_CH * KT_:
                        state["g"] += 1
                        state["stage"] = None

            def newton(S_t, X16, iters, init_ps=None):
                """Warm-started Newton-Schulz: X <- (X(2I-SX) + sym)/2.
                Returns (X16_new, last x_ps PSUM tile)."""
                x_ps = init_ps
                for _ in range(iters):
                    t_ps = pss.tile([128, M], f32, name="nt", tag="sm")
                    nc.tensor.matmul(t_ps[:], S_t, X16, start=True, stop=True)
                    U16 = wpool.tile([128, M], bf16, name="U", tag="U")
                    nc.vector.tensor_sub(U16[:], twoI_s.bitcast(f32), t_ps[:])
                    x_ps = psn.tile([128, M], f32, name="nx", tag="nx")
                    nc.tensor.matmul(x_ps[:], X16, U16[:], start=True, stop=False)
                    nc.tensor.matmul(x_ps[:], U16[:], X16, start=False, stop=True)
                    Xn = wpool.tile([128, M], bf16, name="X", tag="nX")
                    nc.vector.tensor_scalar_mul(Xn[:], x_ps[:], 0.5)
                    X16 = Xn[:]
                return X16, x_ps

            X16 = cpool.tile([128, M], bf16, name="X16w")
            nc.vector.tensor_copy(X16[:], X0_s.bitcast(f32))
            X16 = X16[:]
            D16 = cpool.tile([128, M], bf16, name="D16w")
            nc.vector.tensor_copy(D16[:], D0_s.bitcast(f32))
            D16 = D16[:]
            O11n = O12n = O21n = O22n = None
            P_cur = None

            for d in range(doubles):
                do_rf = refresh(d)
                if do_rf:
                    WT16 = wpool.tile([128, KT_, 2 * M], bf16, name="WT",
                                      tag="WT")
                    if d == 0:
                        # P = Q: WT = P Bt = 0.01 Bt
                        nc.vector.tensor_scalar_mul(
                            WT16[:].rearrange("p k n -> p (k n)"),
                            Bt16[:].rearrange("p k n -> p (k n)"), Q_COST)
                    else:
                        for r in range(2):
                            w_ps = psb.tile([128, N], f32, name=f"w{r}",
                                            tag="big")
                            for k in range(KT_):
                                nc.tensor.matmul(
                                    w_ps[:], Bt_s[:, k, r * M:(r + 1) * M],
                                    P_cur[k], start=(k == 0),
                                    stop=(k == KT_ - 1))
                            W16 = wpool.tile([128, N], bf16, name=f"W{r}",
                                             tag=f"W{r}")
                            nc.vector.tensor_copy(W16[:], w_ps[:])
                            for j in range(KT_):
                                tps = pss.tile([128, 128], bf16, name="wtp",
                                               tag="sm")
                                nc.tensor.transpose(
                                    tps[:], W16[:, j * 128:(j + 1) * 128],
                                    I16[:])
                                nc.vector.tensor_copy(
                                    WT16[:, j, r * M:(r + 1) * M], tps[:])

                    # St blocks: S_rs = sum_k Bt_k[:,r]' WT_k[:,s] (+R~)
                    sblk = {}
                    for r, s in ((0, 0), (0, 1), (1, 0), (1, 1)):
                        s_ps = pss.tile([128, M], f32, name=f"s{r}{s}",
                                        tag="sm")
                        for k in range(KT_):
                            nc.tensor.matmul(
                                s_ps[:], Bt16[:, k, r * M:(r + 1) * M],
                                WT16[:, k, s * M:(s + 1) * M],
                                start=(k == 0), stop=(k == KT_ - 1))
                        if (r, s) == (0, 0):
                            S11f = wpool.tile([128, M], f32, name="S11f",
                                              tag="S11f")
                            nc.vector.tensor_add(S11f[:], R11_s.bitcast(f32),
                                                 s_ps[:])
                        else:
                            St = wpool.tile([128, M], bf16, name=f"S{r}{s}",
                                            tag=f"S{r}{s}")
                            if (r, s) == (1, 1):
                                nc.vector.tensor_add(St[:], Rd_s.bitcast(f32),
                                                     s_ps[:])
                            else:
                                nc.vector.tensor_copy(St[:], s_ps[:])
                            sblk[(r, s)] = St[:]
                    S12_16, S21_16, S22_16 = (sblk[(0, 1)], sblk[(1, 0)],
                                              sblk[(1, 1)])

                    # X ~= S22^-1
                    X16, _ = newton(S22_16, X16, newton_iters(d))
                    # V = X S21 ; VT = V' = S12 X
                    v_ps = pss.tile([128, M], f32, name="v", tag="sm")
                    nc.tensor.matmul(v_ps[:], X16, S21_16, start=True, stop=True)
                    V16 = wpool.tile([128, M], bf16, name="V", tag="V")
                    nc.vector.tensor_copy(V16[:], v_ps[:])
                    vt_ps = pss.tile([128, M], bf16, name="vt", tag="sm")
                    nc.tensor.transpose(vt_ps[:], V16[:], I16[:])
                    VT16 = wpool.tile([128, M], bf16, name="VT", tag="VT")
                    nc.vector.tensor_copy(VT16[:], vt_ps[:])
                    # Dm = S11 - S12 X S21 = S11 - S12 V
                    dm_ps = pss.tile([128, M], f32, name="dm", tag="sm")
                    nc.tensor.matmul(dm_ps[:], S21_16, V16[:], start=True,
                                     stop=True)
                    Dm16 = wpool.tile([128, M], bf16, name="Dm", tag="Dm")
                    nc.vector.tensor_sub(Dm16[:], S11f[:], dm_ps[:])
                    # D ~= Dm^-1
                    D16, dx_ps = newton(Dm16[:], D16, newton_iters(d))
                    # O blocks (negated inverse), f32r
                    O11n = wpool.tile([128, M], f32r, name="O11n", tag="O11n")
                    nc.vector.tensor_scalar_mul(O11n[:], dx_ps[:], -0.5)
                    dvt_ps = pss.tile([128, M], f32, name="dvt", tag="sm")
                    nc.tensor.matmul(dvt_ps[:], D16, VT16[:], start=True,
                                     stop=True)
                    O12n = wpool.tile([128, M], f32r, name="O12n", tag="O12n")
                    nc.vector.tensor_copy(O12n[:], dvt_ps[:])
                    DVtn16 = wpool.tile([128, M], bf16, name="DVtn", tag="DVtn")
                    nc.scalar.mul(DVtn16[:], dvt_ps[:], -1.0)
                    o21_ps = pss.tile([128, M], f32, name="o21", tag="sm")
                    nc.tensor.matmul(o21_ps[:], VT16[:], D16, start=True,
                                     stop=True)
                    O21n = wpool.tile([128, M], f32r, name="O21n", tag="O21n")
                    nc.vector.tensor_copy(O21n[:], o21_ps[:])
                    o22_ps = pss.tile([128, M], f32, name="o22", tag="sm")
                    nc.tensor.matmul(o22_ps[:], VT16[:], DVtn16[:], start=True,
                                     stop=True)
                    O22n = wpool.tile([128, M], f32r, name="O22n", tag="O22n")
                    nc.vector.tensor_sub(O22n[:], o22_ps[:], X16)

                # G = P A2  (d=0: P=Q -> G = 0.01 A2 via DVE)
                G = [None] * KT_
                for i in range(KT_):
                    if d == 0:
                        Gi = ppool.tile([128, N], f32r, name=f"G{i}",
                                        tag=f"G{i}")
                        if i % 2 == 0:
                            nc.vector.tensor_scalar_mul(Gi[:], A2_s[:, i, :],
                                                        Q_COST)
                        else:
                            nc.scalar.mul(Gi[:], A2_s[:, i, :], Q_COST)
                        G[i] = Gi
                    else:
                        g_ps = psb.tile([128, N], f32, name=f"g{i}", tag="big")
                        for k in range(KT_):
                            nc.tensor.matmul(
                                g_ps[:], P_cur[k][:, i * 128:(i + 1) * 128],
                                A2_s[:, k, :], start=(k == 0),
                                stop=(k == KT_ - 1))
                        Gi = ppool.tile([128, N], f32r, name=f"G{i}",
                                        tag=f"G{i}")
                        nc.scalar.copy(Gi[:], g_ps[:])
                        G[i] = Gi
                emit_obs_transposes(3)

                # Yt = Bt'G + C'  (two halves)
                Yt = [None, None]
                for r in range(2):
                    yt_ps = psb.tile([128, N], f32, name=f"yt{r}", tag="big")
                    for k in range(KT_):
                        nc.tensor.matmul(yt_ps[:],
                                         Bt_s[:, k, r * M:(r + 1) * M],
                                         G[k][:], start=(k == 0),
                                         stop=(k == KT_ - 1))
                    Ytr = wpool.tile([128, N], f32r, name=f"Yt{r}",
                                     tag=f"Yt{r}")
                    if r == 0:
                        nc.vector.tensor_add(Ytr[:], CT_s.bitcast(f32),
                                             yt_ps[:])
                    else:
                        nc.vector.tensor_copy(Ytr[:], yt_ps[:])
                    Yt[r] = Ytr

                # z = -St^-1 Yt  via O blocks (already negated)
                z1_ps = psb.tile([128, N], f32, name="z1", tag="big")
                nc.tensor.matmul(z1_ps[:], O11n[:], Yt[0][:], start=True,
                                 stop=False)
                nc.tensor.matmul(z1_ps[:], O21n[:], Yt[1][:], start=False,
                                 stop=True)
                z1n = wpool.tile([128, N], f32r, name="z1n", tag="z1n")
                nc.scalar.copy(z1n[:], z1_ps[:])
                z2_ps = psb.tile([128, N], f32, name="z2", tag="big")
                nc.tensor.matmul(z2_ps[:], O12n[:], Yt[0][:], start=True,
                                 stop=False)
                nc.tensor.matmul(z2_ps[:], O22n[:], Yt[1][:], start=False,
                                 stop=True)
                z2n = wpool.tile([128, N], f32r, name="z2n", tag="z2n")
                nc.vector.tensor_copy(z2n[:], z2_ps[:])

                # P' = Qt + A2'G + Yt1'z1n + Yt2'z2n  (upper blocks + mirror)
                P_new = [None] * KT_
                for i in range(KT_):
                    lo = i * 128 if i in (1, 2) else 0
                    p_ps = psb.tile([128, N], f32, name=f"p{i}", tag="big")
                    for k in range(KT_):
                        nc.tensor.matmul(p_ps[:, lo:N],
                                         A2_s[:, k, i * 128:(i + 1) * 128],
                                         G[k][:, lo:N], start=(k == 0),
                                         stop=False)
                    nc.tensor.matmul(p_ps[:, lo:N],
                                     Yt[0][:, i * 128:(i + 1) * 128],
                                     z1n[:, lo:N], start=False, stop=False)
                    nc.tensor.matmul(p_ps[:, lo:N],
                                     Yt[1][:, i * 128:(i + 1) * 128],
                                     z2n[:, lo:N], start=False, stop=True)
                    Pi = ppool.tile([128, N], f32r, name=f"P{i}", tag=f"P{i}")
                    nc.vector.tensor_add(Pi[:, lo:N],
                                         QT_s[:, i, lo:N].bitcast(f32),
                                         p_ps[:, lo:N])
                    for j in range(i if i in (1, 2) else 0):
                        mps = pss.tile([128, 128], f32r, name="mtp", tag="sm")
                        nc.tensor.transpose(
                            mps[:], P_new[j][:, i * 128:(i + 1) * 128], I_s)
                        if (i + j) % 2 == 0:
                            nc.vector.tensor_copy(Pi[:, j * 128:(j + 1) * 128],
                                                  mps[:])
                        else:
                            nc.scalar.copy(Pi[:, j * 128:(j + 1) * 128],
                                           mps[:])
                    P_new[i] = Pi
                P_cur = [P_new[i][:] for i in range(KT_)]
                emit_obs_transposes(3)

            # ---- final single-step gain on P_48 ----
            # W = B'P (B = second half of Bt)
            wf_ps = psb.tile([128, N], f32, name="wf", tag="big")
            for k in range(KT_):
                nc.tensor.matmul(wf_ps[:], Bt_s[:, k, M:2 * M], P_cur[k],
                                 start=(k == 0), stop=(k == KT_ - 1))
            Wf16 = wpool.tile([128, N], bf16, name="Wf", tag="W0")
            nc.vector.tensor_copy(Wf16[:], wf_ps[:])
            WTf16 = wpool.tile([128, KT_, M], bf16, name="WTf", tag="WT")
            WTfr = wpool.tile([128, KT_, M], f32r, name="WTfr", tag="WTfr")
            for j in range(KT_):
                tps = pss.tile([128, 128], bf16, name="wtp", tag="sm")
                nc.tensor.transpose(tps[:], Wf16[:, j * 128:(j + 1) * 128],
                                    I16[:])
                nc.vector.tensor_copy(WTf16[:, j, :], tps[:])
                nc.scalar.copy(WTfr[:, j, :], tps[:])
            sf_ps = pss.tile([128, M], f32, name="sf", tag="sm")
            for k in range(KT_):
                nc.tensor.matmul(sf_ps[:], WTf16[:, k, :],
                                 Bt16[:, k, M:2 * M], start=(k == 0),
                                 stop=(k == KT_ - 1))
            Sf16 = wpool.tile([128, M], bf16, name="Sf", tag="Sf16")
            nc.vector.tensor_add(Sf16[:], Rd_s.bitcast(f32), sf_ps[:])
            Sfr = wpool.tile([128, M], f32r, name="Sfr", tag="Sfr")
            nc.vector.tensor_add(Sfr[:], Rd_s.bitcast(f32), sf_ps[:])
            X16, fx_ps = newton(Sf16[:], X16, 2)
            XNr = wpool.tile([128, M], f32r, name="XNr", tag="XNr")
            nc.vector.tensor_scalar_mul(XNr[:], fx_ps[:], -0.5)

            # Y = B'P48 A = sum_k WTfr_k' A_k
            yf_ps = psb.tile([128, N], f32, name="yf", tag="big")
            for k in range(KT_):
                nc.tensor.matmul(yf_ps[:], WTfr[:, k, :], A1_s[:, k, :],
                                 start=(k == 0), stop=(k == KT_ - 1))
            Y = wpool.tile([128, N], f32r, name="Y", tag="Yt0")
            nc.vector.tensor_copy(Y[:], yf_ps[:])

            emit_obs_transposes(OGROUPS * OG_CH * KT_)

            # K0 = X Y + one refinement; K0n = -K0'
            k0_ps = psb.tile([128, N], f32, name="k0", tag="big")
            nc.tensor.matmul(k0_ps[:], XNr[:], Y[:], start=True, stop=False)
            K0a = wpool.tile([128, N], f32r, name="K0a", tag="Yt1")
            nc.vector.tensor_scalar_mul(K0a[:], k0_ps[:], -1.0)  # = X Y
            sk_ps = psb.tile([128, N], f32, name="sk", tag="big")
            nc.tensor.matmul(sk_ps[:], Sfr[:], K0a[:], start=True, stop=True)
            E = wpool.tile([128, N], f32r, name="E", tag="z1n")
            nc.vector.tensor_sub(E[:], Y[:].bitcast(f32), sk_ps[:])
            nc.tensor.matmul(k0_ps[:], XNr[:], E[:], start=False, stop=True)
            K0n = wpool.tile([128, N], f32r, name="K0n", tag="z2n")
            nc.vector.tensor_copy(K0n[:], k0_ps[:])   # = -K0'

            K0T = wpool.tile([128, KT_, M], f32r, name="K0T", tag="K0T")
            for j in range(KT_):
                tps = pss.tile([128, 128], f32r, name="ktp", tag="sm")
                nc.tensor.transpose(tps[:], K0n[:, j * 128:(j + 1) * 128], I_s)
                nc.vector.tensor_copy(K0T[:, j, :], tps[:])

            # u0T = -K0 @ obs.T per 512-col group; transpose back; DMA out
            for g in range(SHARD // N):
                u_ps = psb.tile([128, N], f32, name=f"u{g}", tag="big")
                for k in range(KT_):
                    nc.tensor.matmul(u_ps[:], K0T[:, k, :],
                                     obsT[k][:, g * N:(g + 1) * N],
                                     start=(k == 0), stop=(k == KT_ - 1))
                ut = wpool.tile([128, N], f32r, name="UT", tag="UT")
                nc.scalar.copy(ut[:], u_ps[:])
                u0g = spool.tile([128, OG_CH, M], f32, name="u0g", tag="u0g")
                for q in range(KT_):
                    tps2 = pss.tile([128, 128], f32r, name="utp", tag="sm")
                    nc.tensor.transpose(tps2[:], ut[:, q * 128:(q + 1) * 128],
                                        I_s)
                    nc.vector.tensor_copy(u0g[:, q, :], tps2[:])
                nc.sync.dma_start(u0_v[g], u0g[:])
    nc.finalize()
    return nc


def kernel(obs, A, B):
    obs = np.ascontiguousarray(obs, np.float32)
    cblob = build_const_blob(np.asarray(A, np.float32),
                             np.asarray(B, np.float32))
    if "nc" not in _CACHE:
        _CACHE["nc"] = build()
    nc = _CACHE["nc"]
    in_maps = [{"cblob": cblob, "obs": obs[c * SHARD:(c + 1) * SHARD]}
               for c in range(NCORES)]
    res = bass_utils.run_bass_kernel_spmd(nc, in_maps, core_ids=list(range(NCORES)))
    return np.concatenate([r["u0"] for r in res.results], axis=0)


# revision 16
# speedup vs baseline: 1.7964x; 1.0778x over previous
"""Trainium2 Bass kernel for nn_CvxMPC: finite-horizon LQR gain (Riccati
recursion) + batch control computation  u0 = -obs @ K0.T.

Sharding: obs is split along batch across 8 cores (data parallel); A, B and
the entire Riccati recursion are replicated on every core (no collectives).

The 49-step Riccati recursion runs as 24 *composite double steps* (exact
2-step composition: A2 = A^2, Bt = [AB, B], Qt = Q + A'QA,
Rt = blkdiag(R + B'QB, R), C = [A'QB, 0] -- P-independent host-folded
constants), then one single-step gain extraction on P_48:

    per double step (P <- Qt + A2'PA2 - Yt' St^-1 Yt,  St = Rt + Bt'P Bt,
                     Yt = Bt'P A2 + C'):
        W~ = Bt'P;  WT = W~.T (PE transposes) == P Bt
        St blocks S11(+R11), S12, S21, S22(+R) = Bt'WT
        X ~= S22^-1, D ~= (S11 - S12 X S21)^-1  (warm-started bf16
            Newton-Schulz); negated 2x2 block inverse in f32r:
            O11n=-D, O12n=+D S12 X, O21n=+(X S21)D, O22n=-(X + V D V')
        G = P A2;  Yt = Bt'G + C';  z = -St^-1 Yt via O blocks
        P' = Qt + A2'G + Yt'z   (upper blocks + PE-transpose mirrors)
    finally: S = R + B'P48 B, X ~= S^-1, Y = B'P48 A, K0 = X Y (+1 Newton
    refinement), u0.T = -K0 @ obs.T.

Wide matmuls fp32r (1 cyc/row at >=256 out width); narrow 128-wide matmuls
bf16 (fp32r pays 4 cyc/row there); fp32 PSUM accumulate. bf16 constants are
host-packed into the blob and bitcast on device. obs.T is built with PE
transposes interleaved into the Riccati stream (they fill Newton-chain PE
gaps; a reserve fills the final gain-extraction tail).
"""
import numpy as np
import concourse.bacc as bacc
import concourse.mybir as mybir
import concourse.tile as tile
from concourse import bass_utils

f32 = mybir.dt.float32
f32r = mybir.dt.float32r
bf16 = mybir.dt.bfloat16

N = 512          # state dim
M = 128          # control dim
DOUBLES = 24     # composite double steps: P_0=Q -> P_48
import os
I3_NARROW = os.environ.get("I3", "1") == "1"
RF_MODE = os.environ.get("RF", "3")
Q_COST = 0.01
R_COST = 0.01
BATCH = 32768
NCORES = 8
SHARD = BATCH // NCORES          # 4096 rows per core
CHUNKS = SHARD // 128            # 32 [128,512] obs row-chunks per core
KT_ = N // 128                   # 4 k-tiles
OGROUPS = 8                      # obs DMA groups (4 chunks each)
OG_CH = CHUNKS // OGROUPS        # 4
OBS_T_TOTAL = OGROUPS * OG_CH * KT_   # 128 obs transposes
OBS_T_RESERVE = 24               # kept back to fill the final serial tail

X0_SCALE = 44.0  # ~2/(lmin+lmax) of S22_0 = 0.01(I + B'B)
D0_SCALE = 26.0  # ~2/(lmin+lmax) of the d=0 Schur complement


def newton_iters(d):
    if d == 0:
        return 7
    return 2 if d < 3 else 1


def refresh(d):
    """Whether double step d refreshes W~/St/X/D/O (else reuse O blocks)."""
    if RF_MODE == "2":
        return d < 8 or d % 2 == 0
    return d < 8 or (d - 8) % 3 == 0


def r32r_rne(x):
    u = np.ascontiguousarray(x, np.float32).view(np.uint32).copy()
    bias = np.uint32(0x7FF) + ((u >> np.uint32(12)) & np.uint32(1))
    u = (u + bias) & np.uint32(0xFFFFF000)
    return u.view(np.float32)


def bf16_pack(x):
    """[128, W] f32 -> [128, W/2] f32 words holding packed RNE bf16 pairs."""
    u = np.ascontiguousarray(x, np.float32).view(np.uint32)
    bias = np.uint32(0x7FFF) + ((u >> np.uint32(16)) & np.uint32(1))
    h = ((u + bias) >> np.uint32(16)).astype(np.uint32)   # bf16 as uint
    lo, hi = h[:, 0::2], h[:, 1::2]
    return (lo | (hi << np.uint32(16))).view(np.float32)


# ---- constant blob layout (per-partition f32 elements) ----
# section 1 (urgent smalls; unblocks d=0 St/Newton quickly)
OFF_BT = 0                           # Bt=[AB,B] f32r [4 x 256] k-tiles
OFF_2I = OFF_BT + KT_ * 2 * M        # 2*I f32r [128]
OFF_RD = OFF_2I + M                  # R diag [128]
OFF_R11 = OFF_RD + M                 # R + B'QB [128x128]
OFF_I = OFF_R11 + M                  # identity f32r [128]
OFF_X0 = OFF_I + M                   # X0_SCALE*I [128]
OFF_D0 = OFF_X0 + M                  # D0_SCALE*I [128]
SEC1 = OFF_D0 + M
# section 2 (needed within ~10us)
OFF_A2 = SEC1                        # A^2 [4 x 512]
OFF_CT = OFF_A2 + KT_ * N            # C' top = 0.01*B'A [128 x 512]
SEC2 = OFF_CT + N
# section 3 (needed later)
OFF_QT = SEC2                        # Qt row tiles [4 x 512]
OFF_A1 = OFF_QT + KT_ * N            # A [4 x 512] (final step)
CBLOB = OFF_A1 + KT_ * N


def pack_k_tiles(x, width):
    return np.ascontiguousarray(
        x.reshape(KT_, 128, width).transpose(1, 0, 2).reshape(128, KT_ * width))


def build_const_blob(A, B):
    A = np.asarray(A, np.float64)
    B = np.asarray(B, np.float64)
    Q = Q_COST * np.eye(N)
    A2 = A @ A
    Bt = np.concatenate([A @ B, B], axis=1)
    Qt = Q + A.T @ Q @ A
    R11 = R_COST * np.eye(M) + B.T @ Q @ B
    CT = Q_COST * (B.T @ A)
    ident = np.eye(128, dtype=np.float32)

    blob = np.zeros((128, CBLOB), np.float32)
    blob[:, OFF_BT:OFF_BT + KT_ * 2 * M] = pack_k_tiles(
        r32r_rne(Bt.astype(np.float32)), 2 * M)
    blob[:, OFF_2I:OFF_2I + M] = r32r_rne(2.0 * ident)
    blob[:, OFF_RD:OFF_RD + M] = r32r_rne(R_COST * ident)
    blob[:, OFF_R11:OFF_R11 + M] = r32r_rne(R11.astype(np.float32))
    blob[:, OFF_I:OFF_I + M] = ident
    blob[:, OFF_X0:OFF_X0 + M] = r32r_rne(X0_SCALE * ident)
    blob[:, OFF_D0:OFF_D0 + M] = r32r_rne(D0_SCALE * ident)
    blob[:, OFF_A2:OFF_A2 + KT_ * N] = pack_k_tiles(
        r32r_rne(A2.astype(np.float32)), N)
    blob[:, OFF_CT:OFF_CT + N] = r32r_rne(CT.astype(np.float32))
    blob[:, OFF_QT:OFF_QT + KT_ * N] = pack_k_tiles(
        r32r_rne(Qt.astype(np.float32)), N)
    blob[:, OFF_A1:OFF_A1 + KT_ * N] = pack_k_tiles(
        r32r_rne(A.astype(np.float32)), N)
    return blob


_CACHE = {}


def build(doubles=DOUBLES):
    nc = bacc.Bacc(trn_type="TRN2", target_bir_lowering=False)
    cb_d = nc.dram_tensor("cblob", [128, CBLOB], f32r, kind="ExternalInput")
    obs_d = nc.dram_tensor("obs", [SHARD, N], f32r, kind="ExternalInput")
    u0_d = nc.dram_tensor("u0", [SHARD, M], f32, kind="ExternalOutput")
    obs_v = obs_d.ap().rearrange("(g c p) n -> g p c n", p=128, c=OG_CH)
    u0_v = u0_d.ap().rearrange("(g c p) m -> g p c m", p=128, c=OG_CH)

    with tile.TileContext(nc) as tc:
        with tc.tile_pool(name="const", bufs=1) as cpool, \
             tc.tile_pool(name="obsp", bufs=1) as opool, \
             tc.tile_pool(name="stg", bufs=2) as spool, \
             tc.tile_pool(name="work", bufs=2) as wpool, \
             tc.tile_pool(name="pp", bufs=2) as ppool, \
             tc.tile_pool(name="big", bufs=4, space="PSUM") as psb, \
             tc.tile_pool(name="small", bufs=3, space="PSUM") as pss, \
             tc.tile_pool(name="nwt", bufs=1, space="PSUM") as psn:

            cb = cpool.tile([128, CBLOB], f32r, name="cb")
            nc.sync.dma_start(cb[:, 0:SEC1], cb_d.ap()[:, 0:SEC1])
            nc.sync.dma_start(cb[:, SEC1:SEC2], cb_d.ap()[:, SEC1:SEC2])

            twoI_s = cb[:, OFF_2I:OFF_2I + M]
            Rd_s = cb[:, OFF_RD:OFF_RD + M]
            R11_s = cb[:, OFF_R11:OFF_R11 + M]
            I_s = cb[:, OFF_I:OFF_I + M]
            A2_s = cb[:, OFF_A2:OFF_A2 + KT_ * N].rearrange(
                "p (k n) -> p k n", k=KT_)
            Bt_s = cb[:, OFF_BT:OFF_BT + KT_ * 2 * M].rearrange(
                "p (k n) -> p k n", k=KT_)
            CT_s = cb[:, OFF_CT:OFF_CT + N]
            QT_s = cb[:, OFF_QT:OFF_QT + KT_ * N].rearrange(
                "p (k n) -> p k n", k=KT_)
            A1_s = cb[:, OFF_A1:OFF_A1 + KT_ * N].rearrange(
                "p (k n) -> p k n", k=KT_)

            # bf16 working copies of narrow-matmul constants (DVE/Act convert)
            Bt16t = cpool.tile([128, KT_, 2 * M], bf16, name="Bt16")
            nc.vector.tensor_copy(Bt16t[:].rearrange("p k n -> p (k n)"),
                                  cb[:, OFF_BT:OFF_BT + KT_ * 2 * M]
                                  .bitcast(f32))
            Bt16 = Bt16t[:]
            WT016t = cpool.tile([128, KT_, 2 * M], bf16, name="WT016")
            nc.scalar.mul(WT016t[:].rearrange("p k n -> p (k n)"),
                          cb[:, OFF_BT:OFF_BT + KT_ * 2 * M].bitcast(f32),
                          Q_COST)
            WT016 = WT016t[:]
            I16t = cpool.tile([128, M], bf16, name="I16")
            nc.vector.tensor_copy(I16t[:], I_s.bitcast(f32))
            I16 = I16t[:]
            X016t = cpool.tile([128, M], bf16, name="X016")
            nc.scalar.copy(X016t[:], cb[:, OFF_X0:OFF_X0 + M].bitcast(f32))
            X016 = X016t[:]
            D016t = cpool.tile([128, M], bf16, name="D016")
            nc.vector.tensor_copy(D016t[:], cb[:, OFF_D0:OFF_D0 + M]
                                  .bitcast(f32))
            D016 = D016t[:]
            # A2 column-block 3 in bf16 (for the narrow i=3 P' row);
            # converted once A2 lands
            A2C3t = cpool.tile([128, KT_, M], bf16, name="A2C3")
            for k in range(KT_):
                (nc.vector.tensor_copy if k % 2 == 0 else nc.scalar.copy)(
                    A2C3t[:, k, :], A2_s[:, k, 384:512].bitcast(f32))
            A2C3 = A2C3t[:]

            obsT = [opool.tile([128, SHARD], f32r, name=f"obsT{j}")
                    for j in range(KT_)]
            state = {"g": 0, "stage": None, "pos": 0, "done": 0}

            def ensure_stage():
                if state["stage"] is None and state["g"] < OGROUPS:
                    stg = spool.tile([128, OG_CH, N], f32r, name="ostg",
                                     tag="ostg")
                    nc.sync.dma_start(stg[:], obs_v[state["g"]])
                    state["stage"] = stg
                    state["pos"] = 0

            def emit_obs(budget, tail=False):
                for _ in range(budget):
                    if state["g"] >= OGROUPS:
                        return
                    if not tail and \
                            state["done"] >= OBS_T_TOTAL - OBS_T_RESERVE:
                        return
                    ensure_stage()
                    stg = state["stage"]
                    ci, j = divmod(state["pos"], KT_)
                    c = state["g"] * OG_CH + ci
                    tps = pss.tile([128, 128], f32r, name="otp", tag="sm")
                    nc.tensor.transpose(tps[:],
                                        stg[:, ci, j * 128:(j + 1) * 128], I_s)
                    nc.scalar.copy(obsT[j][:, c * 128:(c + 1) * 128], tps[:])
                    state["pos"] += 1
                    state["done"] += 1
                    if state["pos"] == OG_CH * KT_:
                        state["g"] += 1
                        state["stage"] = None

            # queue the first obs group right after the section-2 constants
            ensure_stage()
            nc.sync.dma_start(cb[:, SEC2:CBLOB], cb_d.ap()[:, SEC2:CBLOB])

            def newton(S_t, X16, iters, tail=False, fill=True):
                x_ps = None
                for _ in range(iters):
                    t_ps = pss.tile([128, M], f32, name="nt", tag="sm")
                    nc.tensor.matmul(t_ps[:], S_t, X16, start=True, stop=True)
                    U16 = wpool.tile([128, M], bf16, name="U", tag="U")
                    nc.vector.tensor_sub(U16[:], twoI_s.bitcast(f32), t_ps[:])
                    x_ps = psn.tile([128, M], f32, name="nx", tag="nx")
                    nc.tensor.matmul(x_ps[:], X16, U16[:], start=True,
                                     stop=False)
                    nc.tensor.matmul(x_ps[:], U16[:], X16, start=False,
                                     stop=True)
                    Xn = wpool.tile([128, M], bf16, name="X", tag="nX")
                    nc.vector.tensor_scalar_mul(Xn[:], x_ps[:], 0.5)
                    X16 = Xn[:]
                    if fill:
                        emit_obs(2, tail=tail)
                return X16, x_ps

            X16 = X016
            D16 = D016
            O11n = O12n = O21n = O22n = None
            P_cur = None

            for d in range(doubles):
                do_rf = refresh(d)
                if do_rf:
                    if d == 0:
                        WT16 = WT016
                    else:
                        WT16t = wpool.tile([128, KT_, 2 * M], bf16, name="WT",
                                           tag="WT")
                        for r in range(2):
                            w_ps = psb.tile([128, N], f32, name=f"w{r}",
                                            tag="big")
                            for k in range(KT_):
                                nc.tensor.matmul(
                                    w_ps[:], Bt_s[:, k, r * M:(r + 1) * M],
                                    P_cur[k], start=(k == 0),
                                    stop=(k == KT_ - 1))
                            W16 = wpool.tile([128, N], bf16, name=f"W{r}",
                                             tag=f"W{r}")
                            (nc.vector.tensor_copy if r == 0
                             else nc.scalar.copy)(W16[:], w_ps[:])
                            for j in range(KT_):
                                tps = pss.tile([128, 128], bf16, name="wtp",
                                               tag="sm")
                                nc.tensor.transpose(
                                    tps[:], W16[:, j * 128:(j + 1) * 128],
                                    I16)
                                (nc.vector.tensor_copy if j % 2 == 0
                                 else nc.scalar.copy)(
                                    WT16t[:, j, r * M:(r + 1) * M], tps[:])
                        WT16 = WT16t[:]

                    # St blocks
                    sblk = {}
                    for r, s in ((0, 0), (0, 1), (1, 0), (1, 1)):
                        s_ps = pss.tile([128, M], f32, name=f"s{r}{s}",
                                        tag="sm")
                        for k in range(KT_):
                            nc.tensor.matmul(
                                s_ps[:], Bt16[:, k, r * M:(r + 1) * M],
                                WT16[:, k, s * M:(s + 1) * M],
                                start=(k == 0), stop=(k == KT_ - 1))
                        if (r, s) == (0, 0):
                            S11f = wpool.tile([128, M], f32, name="S11f",
                                              tag="S11f")
                            nc.vector.tensor_add(S11f[:], R11_s.bitcast(f32),
                                                 s_ps[:])
                        else:
                            St = wpool.tile([128, M], bf16, name=f"S{r}{s}",
                                            tag=f"S{r}{s}")
                            if (r, s) == (1, 1):
                                nc.vector.tensor_add(St[:], Rd_s.bitcast(f32),
                                                     s_ps[:])
                            else:
                                nc.scalar.copy(St[:], s_ps[:])
                            sblk[(r, s)] = St[:]
                    S21_16, S22_16 = sblk[(1, 0)], sblk[(1, 1)]

                    X16, _ = newton(S22_16, X16, newton_iters(d), fill=(d > 0))
                    v_ps = pss.tile([128, M], f32, name="v", tag="sm")
                    nc.tensor.matmul(v_ps[:], X16, S21_16, start=True,
                                     stop=True)
                    V16 = wpool.tile([128, M], bf16, name="V", tag="V")
                    nc.vector.tensor_copy(V16[:], v_ps[:])
                    vt_ps = pss.tile([128, M], bf16, name="vt", tag="sm")
                    nc.tensor.transpose(vt_ps[:], V16[:], I16)
                    VT16 = wpool.tile([128, M], bf16, name="VT", tag="VT")
                    nc.scalar.copy(VT16[:], vt_ps[:])
                    dm_ps = pss.tile([128, M], f32, name="dm", tag="sm")
                    nc.tensor.matmul(dm_ps[:], S21_16, V16[:], start=True,
                                     stop=True)
                    Dm16 = wpool.tile([128, M], bf16, name="Dm", tag="Dm")
                    nc.vector.tensor_sub(Dm16[:], S11f[:], dm_ps[:])
                    D16, dx_ps = newton(Dm16[:], D16, newton_iters(d), fill=(d > 0))
                    O11n = wpool.tile([128, M], f32r, name="O11n", tag="O11n")
                    nc.vector.tensor_scalar_mul(O11n[:], dx_ps[:], -0.5)
                    dvt_ps = pss.tile([128, M], f32, name="dvt", tag="sm")
                    nc.tensor.matmul(dvt_ps[:], D16, VT16[:], start=True,
                                     stop=True)
                    O12n = wpool.tile([128, M], f32r, name="O12n", tag="O12n")
                    nc.vector.tensor_copy(O12n[:], dvt_ps[:])
                    DVtn16 = wpool.tile([128, M], bf16, name="DVtn",
                                        tag="DVtn")
                    nc.scalar.mul(DVtn16[:], dvt_ps[:], -1.0)
                    o21_ps = pss.tile([128, M], f32, name="o21", tag="sm")
                    nc.tensor.matmul(o21_ps[:], VT16[:], D16, start=True,
                                     stop=True)
                    O21n = wpool.tile([128, M], f32r, name="O21n", tag="O21n")
                    nc.scalar.copy(O21n[:], o21_ps[:])
                    o22_ps = pss.tile([128, M], f32, name="o22", tag="sm")
                    nc.tensor.matmul(o22_ps[:], VT16[:], DVtn16[:], start=True,
                                     stop=True)
                    O22n = wpool.tile([128, M], f32r, name="O22n", tag="O22n")
                    nc.vector.tensor_sub(O22n[:], o22_ps[:], X16)

                # G = P A2  (d=0: P=Q -> G = 0.01 A2 via DVE/Act)
                G = [None] * KT_
                G16 = [None] * KT_          # [:, 384:512] bf16 slices
                for i in range(KT_):
                    Gi = ppool.tile([128, N], f32r, name=f"G{i}", tag=f"G{i}")
                    Gs = ppool.tile([128, 128], bf16, name=f"G16_{i}",
                                    tag=f"G16_{i}")
                    if d == 0:
                        if i % 2 == 0:
                            nc.vector.tensor_scalar_mul(Gi[:], A2_s[:, i, :],
                                                        Q_COST)
                            nc.scalar.mul(Gs[:], A2_s[:, i, 384:512], Q_COST)
                        else:
                            nc.scalar.mul(Gi[:], A2_s[:, i, :], Q_COST)
                            nc.vector.tensor_scalar_mul(
                                Gs[:], A2_s[:, i, 384:512], Q_COST)
                    else:
                        g_ps = psb.tile([128, N], f32, name=f"g{i}", tag="big")
                        for k in range(KT_):
                            nc.tensor.matmul(
                                g_ps[:], P_cur[k][:, i * 128:(i + 1) * 128],
                                A2_s[:, k, :], start=(k == 0),
                                stop=(k == KT_ - 1))
                        nc.scalar.copy(Gi[:], g_ps[:])
                        nc.vector.tensor_copy(Gs[:], g_ps[:, 384:512])
                    G[i] = Gi
                    G16[i] = Gs
                emit_obs(2)

                # Yt = Bt'G + C'
                Yt = [None, None]
                Yt16 = [None, None]
                for r in range(2):
                    yt_ps = psb.tile([128, N], f32, name=f"yt{r}", tag="big")
                    for k in range(KT_):
                        nc.tensor.matmul(yt_ps[:],
                                         Bt_s[:, k, r * M:(r + 1) * M],
                                         G[k][:], start=(k == 0),
                                         stop=(k == KT_ - 1))
                    Ytr = wpool.tile([128, N], f32r, name=f"Yt{r}",
                                     tag=f"Yt{r}")
                    Yts = wpool.tile([128, 128], bf16, name=f"Yt16_{r}",
                                     tag=f"Yt16_{r}")
                    if r == 0:
                        nc.vector.tensor_add(Ytr[:], CT_s.bitcast(f32),
                                             yt_ps[:])
                        nc.vector.tensor_add(Yts[:],
                                             CT_s[:, 384:512].bitcast(f32),
                                             yt_ps[:, 384:512])
                    else:
                        nc.scalar.copy(Ytr[:], yt_ps[:])
                        nc.vector.tensor_copy(Yts[:], yt_ps[:, 384:512])
                    Yt[r] = Ytr
                    Yt16[r] = Yts

                # z = -St^-1 Yt via O blocks
                z1_ps = psb.tile([128, N], f32, name="z1", tag="big")
                nc.tensor.matmul(z1_ps[:], O11n[:], Yt[0][:], start=True,
                                 stop=False)
                nc.tensor.matmul(z1_ps[:], O21n[:], Yt[1][:], start=False,
                                 stop=True)
                z1n = wpool.tile([128, N], f32r, name="z1n", tag="z1n")
                nc.scalar.copy(z1n[:], z1_ps[:])
                z1n16 = wpool.tile([128, 128], bf16, name="z1n16", tag="z116")
                nc.vector.tensor_copy(z1n16[:], z1_ps[:, 384:512])
                z2_ps = psb.tile([128, N], f32, name="z2", tag="big")
                nc.tensor.matmul(z2_ps[:], O12n[:], Yt[0][:], start=True,
                                 stop=False)
                nc.tensor.matmul(z2_ps[:], O22n[:], Yt[1][:], start=False,
                                 stop=True)
                z2n = wpool.tile([128, N], f32r, name="z2n", tag="z2n")
                nc.vector.tensor_copy(z2n[:], z2_ps[:])
                z2n16 = wpool.tile([128, 128], bf16, name="z2n16", tag="z216")
                nc.scalar.add(z2n16[:], z2_ps[:, 384:512], 0.0)

                # P' = Qt + A2'G + Yt1'z1n + Yt2'z2n
                # i=0: full width; i=1,2: cols >= i*128 (mirror below);
                # i=3: only the diagonal block, in narrow bf16 matmuls.
                P_new = [None] * KT_
                for i in range(KT_):
                    lo = 0 if i == 0 else (i * 128 if (i < 3 or I3_NARROW)
                                           else 0)
                    p_ps = psb.tile([128, N], f32, name=f"p{i}", tag="big")
                    if i < 3 or not I3_NARROW:
                        for k in range(KT_):
                            nc.tensor.matmul(p_ps[:, lo:N],
                                             A2_s[:, k, i * 128:(i + 1) * 128],
                                             G[k][:, lo:N], start=(k == 0),
                                             stop=False)
                        nc.tensor.matmul(p_ps[:, lo:N],
                                         Yt[0][:, i * 128:(i + 1) * 128],
                                         z1n[:, lo:N], start=False, stop=False)
                        nc.tensor.matmul(p_ps[:, lo:N],
                                         Yt[1][:, i * 128:(i + 1) * 128],
                                         z2n[:, lo:N], start=False, stop=True)
                    else:
                        for k in range(KT_):
                            nc.tensor.matmul(p_ps[:, lo:N], A2C3[:, k, :],
                                             G16[k][:], start=(k == 0),
                                             stop=False)
                        nc.tensor.matmul(p_ps[:, lo:N], Yt16[0][:],
                                         z1n16[:], start=False, stop=False)
                        nc.tensor.matmul(p_ps[:, lo:N], Yt16[1][:],
                                         z2n16[:], start=False, stop=True)
                    Pi = ppool.tile([128, N], f32r, name=f"P{i}", tag=f"P{i}")
                    nc.vector.tensor_add(Pi[:, lo:N],
                                         QT_s[:, i, lo:N].bitcast(f32),
                                         p_ps[:, lo:N])
                    for j in range(i if (i < 3 or I3_NARROW) else 0):
                        mps = pss.tile([128, 128], f32r, name="mtp", tag="sm")
                        nc.tensor.transpose(
                            mps[:], P_new[j][:, i * 128:(i + 1) * 128], I_s)
                        if (i + j) % 2 == 0:
                            nc.vector.tensor_copy(Pi[:, j * 128:(j + 1) * 128],
                                                  mps[:])
                        else:
                            nc.scalar.copy(Pi[:, j * 128:(j + 1) * 128],
                                           mps[:])
                    P_new[i] = Pi
                P_cur = [P_new[i][:] for i in range(KT_)]
                emit_obs(2)

            # ---- final single-step gain on P_48 ----
            wf_ps = psb.tile([128, N], f32, name="wf", tag="big")
            for k in range(KT_):
                nc.tensor.matmul(wf_ps[:], Bt_s[:, k, M:2 * M], P_cur[k],
                                 start=(k == 0), stop=(k == KT_ - 1))
            Wf16 = wpool.tile([128, N], bf16, name="Wf", tag="W0")
            nc.vector.tensor_copy(Wf16[:], wf_ps[:])
            emit_obs(4, tail=True)
            WTf16 = wpool.tile([128, KT_, M], bf16, name="WTf", tag="WT")
            WTfr = wpool.tile([128, KT_, M], f32r, name="WTfr", tag="WTfr")
            for j in range(KT_):
                tps = pss.tile([128, 128], bf16, name="wtp", tag="sm")
                nc.tensor.transpose(tps[:], Wf16[:, j * 128:(j + 1) * 128],
                                    I16)
                nc.vector.tensor_copy(WTf16[:, j, :], tps[:])
                nc.scalar.copy(WTfr[:, j, :], tps[:])
            sf_ps = pss.tile([128, M], f32, name="sf", tag="sm")
            for k in range(KT_):
                nc.tensor.matmul(sf_ps[:], WTf16[:, k, :],
                                 Bt16[:, k, M:2 * M], start=(k == 0),
                                 stop=(k == KT_ - 1))
            Sf16 = wpool.tile([128, M], bf16, name="Sf", tag="Sf16")
            nc.vector.tensor_add(Sf16[:], Rd_s.bitcast(f32), sf_ps[:])
            Sfr = wpool.tile([128, M], f32r, name="Sfr", tag="Sfr")
            nc.vector.tensor_add(Sfr[:], Rd_s.bitcast(f32), sf_ps[:])

            # Y = B'P48 A = sum_k WTfr_k' A_k (emitted before the final
            # Newton so the wide matmuls overlap its serial chain)
            yf_ps = psb.tile([128, N], f32, name="yf", tag="big")
            for k in range(KT_):
                nc.tensor.matmul(yf_ps[:], WTfr[:, k, :], A1_s[:, k, :],
                                 start=(k == 0), stop=(k == KT_ - 1))
            Y = wpool.tile([128, N], f32r, name="Y", tag="Yt0")
            nc.vector.tensor_copy(Y[:], yf_ps[:])

            X16, fx_ps = newton(Sf16[:], X16, 2, tail=True)
            XNr = wpool.tile([128, M], f32r, name="XNr", tag="XNr")
            nc.vector.tensor_scalar_mul(XNr[:], fx_ps[:], -0.5)

            # K0 = X Y + one refinement; K0n = -K0'
            k0_ps = psb.tile([128, N], f32, name="k0", tag="big")
            nc.tensor.matmul(k0_ps[:], XNr[:], Y[:], start=True, stop=False)
            K0a = wpool.tile([128, N], f32r, name="K0a", tag="Yt1")
            nc.vector.tensor_scalar_mul(K0a[:], k0_ps[:], -1.0)  # = X Y
            emit_obs(4, tail=True)
            sk_ps = psb.tile([128, N], f32, name="sk", tag="big")
            nc.tensor.matmul(sk_ps[:], Sfr[:], K0a[:], start=True, stop=True)
            E = wpool.tile([128, N], f32r, name="E", tag="z1n")
            nc.vector.tensor_sub(E[:], Y[:].bitcast(f32), sk_ps[:])
            emit_obs(4, tail=True)
            nc.tensor.matmul(k0_ps[:], XNr[:], E[:], start=False, stop=True)
            K0n = wpool.tile([128, N], f32r, name="K0n", tag="z2n")
            nc.vector.tensor_copy(K0n[:], k0_ps[:])   # = -K0'

            K0T = wpool.tile([128, KT_, M], f32r, name="K0T", tag="K0T")
            for j in range(KT_):
                tps = pss.tile([128, 128], f32r, name="ktp", tag="sm")
                nc.tensor.transpose(tps[:], K0n[:, j * 128:(j + 1) * 128], I_s)
                nc.vector.tensor_copy(K0T[:, j, :], tps[:])
                emit_obs(3, tail=True)
            emit_obs(OBS_T_TOTAL, tail=True)

            # u0T = -K0 @ obs.T per 512-col group; transpose back; DMA out
            for g in range(SHARD // N):
                u_ps = psb.tile([128, N], f32, name=f"u{g}", tag="big")
                for k in range(KT_):
                    nc.tensor.matmul(u_ps[:], K0T[:, k, :],
                                     obsT[k][:, g * N:(g + 1) * N],
                                     start=(k == 0), stop=(k == KT_ - 1))
                ut = wpool.tile([128, N], f32r, name="UT", tag="UT")
                nc.scalar.copy(ut[:], u_ps[:])
                u0g = spool.tile([128, OG_CH, M], f32, name="u0g", tag="u0g")
                for q in range(KT_):
                    tps2 = pss.tile([128, 128], f32r, name="utp", tag="sm")
                    nc.tensor.transpose(tps2[:], ut[:, q * 128:(q + 1) * 128],
                                        I_s)
                    nc.vector.tensor_copy(u0g[:, q, :], tps2[:])
                nc.sync.dma_start(u0_v[g], u0g[:])
    nc.finalize()
    return nc


def kernel(obs, A, B):
    obs = np.ascontiguousarray(obs, np.float32)
    cblob = build_const_blob(np.asarray(A, np.float32),
                             np.asarray(B, np.float32))
    if "nc" not in _CACHE:
        _CACHE["nc"] = build()
    nc = _CACHE["nc"]
    in_maps = [{"cblob": cblob, "obs": obs[c * SHARD:(c + 1) * SHARD]}
               for c in range(NCORES)]
    res = bass_utils.run_bass_kernel_spmd(nc, in_maps, core_ids=list(range(NCORES)))
    return np.concatenate([r["u0"] for r in res.results], axis=0)
